# revision 21
# baseline (speedup 1.0000x reference)
"""Trainium2 Bass kernel for nn_Matcher (retrieval_knn attention), fp8 edition.

Math (per object o, with S=1 batch):
  logits[b,n] = (keys[o,:,b] . q_in[:,n]) / sqrt(Dk)
  p           = softmax_b(logits)
  mem[v,n]    = sum_b values[o,v,b] p[b,n]
  maskmem[n]  = sum_b masks[o,b] p[b,n]
  out[o]      = concat([mem, q_out * maskmem], axis=0)   # [1024, n]

Sharding: 8 cores = 4 objects x 2 query halves (n in [0,1800) / [1800,3600)).
No cross-core communication.

Performance design (vs the fp32r baseline):
  * All matmuls run in fp8 e4m3 with MatmulPerfMode.DoubleRow: each PE
    instruction contracts TWO 128-deep k-tiles at 0.5 cycles per output
    column (2 fp8 weights per PE cell) -- 4x the fp32r matmul rate.
      - mm1 (logits): d=128 contraction split as two 64-deep tiles.
      - mm2 (values/mask/denominator): bank chunks processed in pairs.
  * exp on ACT is then the bottleneck: it is fused over chunk PAIRS
    ([128,2,nw] PSUM -> fp8 SBUF in one instruction) to amortize the
    per-instruction SBUF/PSUM access overhead.  The fp8 exp output feeds
    mm2 directly.
  * Accuracy: the output norm is dominated by the q_out*maskmem half
    (entries ~0.5 std vs ~0.02 std for the mem half), and maskmem/denom
    are positive-weighted averages where fp8 noise averages down; measured
    rel err is ~2e-3 against the 2e-2 gate.
  * exp is computed as exp(s/sqrt(Dk) - 2) to keep values in e4m3 range;
    the uniform e^-2 factor cancels exactly in the softmax normalization.

Per n-chunk (nw<=512) pipeline, ACT-paced, PSUM = 8 banks exactly:
  s-pairs (2 tiles x 2 banks, double buffered)  mm1 -> exp
  accA0, accA1 (mem rows 0..255)  + md (maskraw+denom, M=2)  accumulate
    per pair, chasing exp;
  accB (1 bank): mem rows 256..383 then 384..511 as two sub-sweeps that
    re-read the previous n-chunk's fp8 exp tiles, interleaved into the
    next chunk's pair loop so ACT never stalls.
"""

import sys

sys.path.insert(0, "/opt/trn_rl_repo")

import numpy as np

OBJ_N, D_KEY, D_VAL, BANK_N, N_Q = 4, 128, 512, 7200, 3600
N_CORES = 8
N_HALF = N_Q // 2            # 1800 queries per core
P = 128
NB = 58                      # bank chunks, padded: 58*128 = 7424
B_PAD = NB * P
NPAIR = NB // 2              # 29
VHAT_W = D_VAL + 2           # 514: values^T | mask | ones
VHAT_WP = 576                # padded: dual-fp8 ldweights needs 64B-aligned
                             # k-tile stride (512 ok, 514 rejected by walrus)
SCALE = 1.0 / float(np.sqrt(D_KEY))
EXP_BIAS = -2.0              # exp(s*SCALE - 2): cancels in normalization
N_CHUNKS = [(0, 512), (512, 512), (1024, 512), (1536, 264)]  # sum = 1800
NJ = len(N_CHUNKS)

_CACHE = {}


def _build(reps=1, bench=False, reload_in_rep=True):
    import concourse.bacc as bacc
    import concourse.mybir as mybir
    import concourse.tile as tile

    f32 = mybir.dt.float32
    f32r = mybir.dt.float32r
    f8 = mybir.dt.float8e4
    Exp = mybir.ActivationFunctionType.Exp
    DR = mybir.MatmulPerfMode.DoubleRow

    nc = bacc.Bacc("TRN2", target_bir_lowering=False, debug=False)

    ikind = {} if bench else {"kind": "ExternalInput"}
    okind = {} if bench else {"kind": "ExternalOutput"}
    consts_d = nc.dram_tensor("consts", [2, P], f32r, kind="ExternalInput")
    ebias_d = nc.dram_tensor("ebias", [P, 1], f32, kind="ExternalInput")
    keys_d = nc.dram_tensor("keys", [64, NB, 2, P], f8, **ikind)
    vhat_d = nc.dram_tensor("vhat", [P, NB, VHAT_WP], f8, **ikind)
    qin_d = nc.dram_tensor("qin", [64, 2, N_HALF], f8, **ikind)
    qout_d = nc.dram_tensor("qout", [D_VAL, N_HALF], f32, **ikind)
    out_d = nc.dram_tensor("out", [2 * D_VAL, N_HALF], f32, **okind)
    if bench:
        dout_d = nc.dram_tensor("dout", [1, P], f32, kind="ExternalOutput")

    qout_ap = qout_d.ap().rearrange("(c p) n -> p c n", p=P)        # [128, 4, 1800]
    out_ap = out_d.ap().rearrange("(r p) n -> p r n", p=P)          # [128, 8, 1800]

    with tile.TileContext(nc) as tc:
        with (
            tc.tile_pool(name="persist", bufs=1) as persist,
            tc.tile_pool(name="qin_p", bufs=2) as qin_p,
            tc.tile_pool(name="qout_p", bufs=2) as qout_p,
            tc.tile_pool(name="e_p", bufs=2 * NPAIR) as e_p,
            tc.tile_pool(name="row_p", bufs=2) as row_p,
            tc.tile_pool(name="bcsb_p", bufs=2) as bcsb_p,
            tc.tile_pool(name="out_p", bufs=2) as out_p,
            tc.tile_pool(name="s_ps", bufs=2, space="PSUM") as s_ps,
            tc.tile_pool(name="accA_ps", bufs=1, space="PSUM") as accA_ps,
            tc.tile_pool(name="accB_ps", bufs=1, space="PSUM") as accB_ps,
            tc.tile_pool(name="mdbc_ps", bufs=1, space="PSUM") as mdbc_ps,
        ):
            # Persistent operands
            keys_sb = persist.tile([64, NB, 2, P], f8)
            vhat_sb = persist.tile([P, NB, VHAT_WP], f8)
            ebias = persist.tile([P, 1], f32)
            # Warm the ACT exp table so the first real exp doesn't pay the
            # ~2.7us ACT_TABLE_LOAD on the critical path.
            warm = persist.tile([1, 1], f32)
            nc.vector.memset(warm[:], 0.0)
            nc.scalar.activation(warm[:], warm[:], Exp, scale=1.0)
            sel2 = persist.tile([2, P], f32r)  # row0=0, row1=1 (selects denom)
            ones_sb = persist.tile([1, P], f32r)  # row of ones (mask broadcast)
            ones_col = ones_sb[:]
            consts_loaded = [False]

            def bulk_load(first_rep):
                # SP HWDGE queue: qin first (mm1-critical), qout later.
                # Pool SWDGE queue: keys/vhat groups, with the small consts
                # interleaved after the first group so they don't delay the
                # first matmul's data.
                n0_0, nw_0 = N_CHUNKS[0]
                qin_t0 = qin_p.tile([64, 2, nw_0], f8, tag="qin", name="qin_t0")
                nc.sync.dma_start(qin_t0[:], qin_d.ap()[:, :, n0_0:n0_0 + nw_0])
                nc.sync.dma_start(ebias[:], ebias_d.ap()[:, :])
                if first_rep:
                    # all keys up-front (0.95 MB total): exp never waits for
                    # mm1 data; vhat (consumed later by the mem/md matmuls,
                    # which may lag) streams in behind it
                    for g0, g1 in ((0, 2), (2, 16), (16, NB)):
                        nc.gpsimd.dma_start(keys_sb[:, g0:g1], keys_d.ap()[:, g0:g1])
                    g0 = 0
                    for gi, gsz in enumerate([4] * NB):
                        if g0 >= NB:
                            break
                        g1 = min(g0 + gsz, NB)
                        nc.gpsimd.dma_start(vhat_sb[:, g0:g1, :],
                                            vhat_d.ap()[:, g0:g1, :])
                        g0 = g1
                        if gi == 0 and not consts_loaded[0]:
                            nc.gpsimd.dma_start(sel2[:], consts_d.ap()[:, :])
                            nc.gpsimd.dma_start(ones_sb[:], consts_d.ap()[1:2, :])
                            consts_loaded[0] = True
                qout_t0 = qout_p.tile([P, D_VAL // P, nw_0], f32,
                                      tag="qout", name="qout_t0")
                return qin_t0, qout_t0

            def mm1_pair(sp, keys_c0, keys_c1, qin_t, nw):
                nc.tensor.matmul(sp[:, 0, :nw], keys_sb[:, keys_c0], qin_t[:],
                                 start=True, stop=True, perf_mode=DR)
                nc.tensor.matmul(sp[:, 1, :nw], keys_sb[:, keys_c1], qin_t[:],
                                 start=True, stop=True, perf_mode=DR)

            HOIST = 2  # exp-pairs of chunk j+1 emitted before chunk j's
                       # normalization block, so ACT never idles across the
                       # chunk boundary (= s_ps bufs)

            for _rep in range(reps):
                qin_t0, qout_t0 = bulk_load(reload_in_rep or _rep == 0)

                # Pipeline state carried across n-chunks
                prev = None  # (e8_tiles, rb, mn, qout_t, n0, nw) of chunk j-1
                nxt = None   # (qin_t, qout_t, e8_tiles) prefetched for chunk j+1

                def emit_front(e8_tiles, qin_t, pp, nw):
                    c0, c1 = 2 * pp, 2 * pp + 1
                    sp = s_ps.tile([P, 2, 512], f32, tag="s", name="sp")
                    e8 = e_p.tile([P, 2, nw], f8, tag="e8", name="e8")
                    if pp == NPAIR - 1:
                        # chunk 57 is pure bank padding (keys=0, vhat=0): skip
                        # its mm1+exp; zero e8 half 1 so the DR matmuls read a
                        # finite value (x0 contribution from the zero vhat)
                        nc.tensor.matmul(sp[:, 0, :nw], keys_sb[:, c0], qin_t[:],
                                         start=True, stop=True, perf_mode=DR)
                        nc.vector.memset(e8[:, 1, :], 0.0)
                        nc.scalar.activation(e8[:, 0, :], sp[:, 0, :nw], Exp,
                                             scale=SCALE, bias=ebias[:])
                    else:
                        mm1_pair(sp, c0, c1, qin_t, nw)
                        nc.scalar.activation(e8[:, :, :], sp[:, :, :nw], Exp,
                                             scale=SCALE, bias=ebias[:])
                    e8_tiles.append(e8)

                FRONT0 = 4  # chunk-0 fronts emitted before the acc loop so
                            # the pair-0 mem matmuls (gated on the vhat DMA)
                            # cannot stall the mm1->exp feed on the in-order PE

                for j in range(NJ):
                    is_real = True
                    if is_real:
                        n0, nw = N_CHUNKS[j]
                        if j == 0:
                            qin_t, qout_t = qin_t0, qout_t0
                            e8_tiles = []
                            for fp in range(FRONT0):
                                emit_front(e8_tiles, qin_t, fp, nw)
                        else:
                            qin_t, qout_t, e8_tiles = nxt
                            nxt = None
                        accA = [accA_ps.tile([P, nw], f32, tag=f"accA{m}",
                                             name=f"accA{m}") for m in (0, 1)]
                        md_acc = mdbc_ps.tile([2, nw], f32, tag="mdbc", name="md_acc")

                    if prev is not None:
                        pe8, prb, pmn, pqout_t, pn0, pnw = prev
                        accB2 = accB_ps.tile([P, pnw], f32, tag="accB", name="accB2")

                    # ---- pair loop ----
                    for pp in range(NPAIR):
                        c0, c1 = 2 * pp, 2 * pp + 1
                        if is_real:
                            if (pp >= FRONT0 if j == 0 else pp >= HOIST):
                                emit_front(e8_tiles, qin_t, pp, nw)
                            if pp == 10 and j == 0:
                                nc.sync.dma_start(
                                    qout_t[:], qout_ap[:, :, n0:n0 + nw])
                            if pp == 20 and j + 1 < NJ:
                                # prefetch next chunk's query tiles
                                nn0, nnw = N_CHUNKS[j + 1]
                                nqin = qin_p.tile([64, 2, nnw], f8, tag="qin")
                                nc.sync.dma_start(
                                    nqin[:], qin_d.ap()[:, :, nn0:nn0 + nnw])
                                nqout = qout_p.tile([P, D_VAL // P, nnw], f32,
                                                    tag="qout")
                                nc.sync.dma_start(
                                    nqout[:], qout_ap[:, :, nn0:nn0 + nnw])
                                nxt = (nqin, nqout, [])
                            e8 = e8_tiles[pp]
                            nc.tensor.matmul(
                                md_acc[:, :], vhat_sb[:, c0:c0 + 2, D_VAL:D_VAL + 2],
                                e8[:, :, :], start=(pp == 0), stop=(pp == NPAIR - 1),
                                perf_mode=DR)
                            for m in (0, 1):
                                nc.tensor.matmul(
                                    accA[m][:, :],
                                    vhat_sb[:, c0:c0 + 2, m * P:(m + 1) * P],
                                    e8[:, :, :],
                                    start=(pp == 0), stop=(pp == NPAIR - 1),
                                    perf_mode=DR)
                        if prev is not None:
                            # sweep B of chunk j-1, compressed to 4 matmuls per
                            # pair: mem rows 256..383 during pairs 0..7, rows
                            # 384..511 during pairs 8..15.  Finishing early
                            # frees the accB bank so the LAST chunk can run its
                            # own B2 sweep inside this loop (avoiding a long
                            # un-overlapped tail after the final exp).
                            if pp < 8:
                                for c in range(4 * pp, min(4 * pp + 4, NPAIR)):
                                    nc.tensor.matmul(
                                        accB2[:, :],
                                        vhat_sb[:, 2 * c:2 * c + 2, 2 * P:3 * P],
                                        pe8[c][:, :, :],
                                        start=(c == 0), stop=(c == NPAIR - 1),
                                        perf_mode=DR)
                                if pp == 7:
                                    o_t = out_p.tile([P, pnw], f32, tag="out")
                                    nc.vector.tensor_mul(o_t[:], accB2[:], prb[:])
                                    nc.sync.dma_start(
                                        out_ap[:, 2, pn0:pn0 + pnw], o_t[:])
                                    accB3 = accB_ps.tile([P, pnw], f32, tag="accB",
                                                         name="accB3")
                            elif pp < 16:
                                for c in range(4 * (pp - 8), min(4 * (pp - 8) + 4,
                                                                 NPAIR)):
                                    nc.tensor.matmul(
                                        accB3[:, :],
                                        vhat_sb[:, 2 * c:2 * c + 2, 3 * P:4 * P],
                                        pe8[c][:, :, :],
                                        start=(c == 0), stop=(c == NPAIR - 1),
                                        perf_mode=DR)
                                if pp == 15:
                                    o_t = out_p.tile([P, pnw], f32, tag="out")
                                    nc.vector.tensor_mul(o_t[:], accB3[:], prb[:])
                                    nc.sync.dma_start(
                                        out_ap[:, 3, pn0:pn0 + pnw], o_t[:])
                            elif is_real and j == NJ - 1 and pp >= 17:
                                # last chunk: own B2 sweep inline, trailing the
                                # exp stream (only pairs already exp'd: c <= pp)
                                if pp == 17:
                                    accB2o = accB_ps.tile([P, nw], f32, tag="accB",
                                                          name="accB2o")
                                    b2o_cur = 0
                                for c in range(b2o_cur,
                                               min(b2o_cur + 3, pp + 1, NPAIR)):
                                    nc.tensor.matmul(
                                        accB2o[:, :],
                                        vhat_sb[:, 2 * c:2 * c + 2, 2 * P:3 * P],
                                        e8_tiles[c][:, :, :],
                                        start=(c == 0), stop=(c == NPAIR - 1),
                                        perf_mode=DR)
                                b2o_cur = min(b2o_cur + 3, pp + 1, NPAIR)

                    # ---- post-pair block ----
                    if j + 1 < NJ:
                        # hoisted fronts of chunk j+1: keep ACT busy while
                        # the normalization block below stalls PE briefly
                        nnw = N_CHUNKS[j + 1][1]
                        for hp in range(HOIST):
                            emit_front(nxt[2], nxt[0], hp, nnw)
                    # Normalization: md_acc rows 0/1 = [maskraw, denom].
                    md2 = row_p.tile([2, nw], f32r, tag="md2")
                    nc.vector.tensor_copy(md2[:], md_acc[0:2, :])
                    db = mdbc_ps.tile([P, nw], f32, tag="mdbc", name="db_ps")
                    nc.tensor.matmul(db[:], sel2[:], md2[:], start=True, stop=True)
                    rb = bcsb_p.tile([P, nw], f32, tag="rb")
                    nc.vector.reciprocal(rb[:], db[:])

                    mb = mdbc_ps.tile([P, nw], f32, tag="mdbc", name="mb_ps")
                    nc.tensor.matmul(mb[:], ones_col, md2[0:1, :],
                                     start=True, stop=True)

                    if j == NJ - 1:
                        # drain the inline B2 sweep as soon as rb exists
                        o_t = out_p.tile([P, nw], f32, tag="out")
                        nc.vector.tensor_mul(o_t[:], accB2o[:], rb[:])
                        nc.sync.dma_start(out_ap[:, 2, n0:n0 + nw], o_t[:])

                    # drain sweep A accs (mem rows 0..255): one fused DMA
                    st01 = out_p.tile([P, 2, nw], f32, tag="st01")
                    for m in (0, 1):
                        nc.vector.tensor_mul(st01[:, m, :], accA[m][:], rb[:])
                    nc.sync.dma_start(out_ap[:, 0:2, n0:n0 + nw], st01[:])

                    if j == NJ - 1:
                        # B3 sweep of the last chunk in accA0's slot (freed by
                        # the st01 drain above): runs on PE in parallel with
                        # the DVE mn/st47 path below
                        accB3o = accA_ps.tile([P, nw], f32, tag="accA0",
                                              name="accB3o")
                        for c in range(NPAIR):
                            nc.tensor.matmul(
                                accB3o[:, :],
                                vhat_sb[:, 2 * c:2 * c + 2, 3 * P:4 * P],
                                e8_tiles[c][:, :, :],
                                start=(c == 0), stop=(c == NPAIR - 1),
                                perf_mode=DR)

                    mn = bcsb_p.tile([P, nw], f32, tag="mn")
                    nc.vector.tensor_mul(mn[:], mb[:], rb[:])
                    # q_out * maskmem rows: two 2-row DMAs (first transfer
                    # overlaps the second pair's muls)
                    st47 = out_p.tile([P, 4, nw], f32, tag="st47")
                    for m in range(2):
                        nc.vector.tensor_mul(st47[:, m, :], qout_t[:, m, :], mn[:])
                    nc.sync.dma_start(out_ap[:, 4:6, n0:n0 + nw], st47[:, 0:2, :])
                    for m in (2, 3):
                        nc.vector.tensor_mul(st47[:, m, :], qout_t[:, m, :], mn[:])
                    nc.sync.dma_start(out_ap[:, 6:8, n0:n0 + nw], st47[:, 2:4, :])

                    if j == NJ - 1:
                        o_t = out_p.tile([P, nw], f32, tag="out")
                        nc.vector.tensor_mul(o_t[:], accB3o[:], rb[:])
                        nc.sync.dma_start(out_ap[:, 3, n0:n0 + nw], o_t[:])
                        prev = None
                    else:
                        prev = (e8_tiles, rb, mn, qout_t, n0, nw)

            if bench:
                dsb = persist.tile([1, P], f32)
                nc.vector.tensor_copy(dsb[:], ones_sb[:])
                nc.sync.dma_start(dout_d.ap()[:, :], dsb[:])

    nc.compile()
    return nc


def _get_nc():
    if "nc" not in _CACHE:
        _CACHE["nc"] = _build()
    return _CACHE["nc"]


def _get_runner():
    """Build the multi-core PJRT runner once (mirrors bass2jax.run_bass_via_pjrt)."""
    if "runner" in _CACHE:
        return _CACHE["runner"]
    import jax
    from jax.sharding import Mesh, PartitionSpec
    from jax.experimental.shard_map import shard_map
    import concourse.mybir as mybir
    from concourse import bass2jax
    from concourse.bass2jax import _bass_exec_p, install_neuronx_cc_hook

    nc = _get_nc()
    install_neuronx_cc_hook()
    partition_name = nc.partition_id_tensor.name if nc.partition_id_tensor else None
    in_names, out_names, out_avals = [], [], []
    for alloc in nc.m.functions[0].allocations:
        if not isinstance(alloc, mybir.MemoryLocationSet):
            continue
        name = alloc.memorylocations[0].name
        if alloc.kind == "ExternalInput":
            if name != partition_name:
                in_names.append(name)
        elif alloc.kind == "ExternalOutput":
            out_names.append(name)
            out_avals.append(jax.core.ShapedArray(
                tuple(alloc.tensor_shape), mybir.dt.np(alloc.dtype)))
    n_params = len(in_names)
    zero_outs = [np.zeros(a.shape, a.dtype) for a in out_avals]
    all_in_names = list(in_names) + list(out_names)
    if partition_name is not None:
        all_in_names.append(partition_name)

    def _body(*args):
        operands = list(args)
        if partition_name is not None:
            operands.append(bass2jax.partition_id_tensor())
        outs = _bass_exec_p.bind(
            *operands,
            out_avals=tuple(out_avals),
            in_names=tuple(all_in_names),
            out_names=tuple(out_names),
            lowering_input_output_aliases=(),
            sim_require_finite=True,
            sim_require_nnan=True,
            nc=nc,
        )
        return tuple(outs)

    try:
        devices = jax.devices("axon")
    except Exception:
        devices = [d for d in jax.devices() if d.platform != "cpu"] or jax.devices()
    devices = devices[:N_CORES]
    assert len(devices) >= N_CORES, f"need {N_CORES} cores, got {len(devices)}"
    mesh = Mesh(np.asarray(devices), ("core",))
    n_io = n_params + len(out_names)
    fn = jax.jit(
        shard_map(_body, mesh=mesh,
                  in_specs=(PartitionSpec("core"),) * n_io,
                  out_specs=(PartitionSpec("core"),) * len(out_names),
                  check_rep=False),
        keep_unused=True)

    def run(in_maps):
        concat_in = [
            np.concatenate([np.asarray(m[name]) for m in in_maps], axis=0)
            for name in in_names
        ]
        concat_zero = [
            np.zeros((N_CORES * z.shape[0], *z.shape[1:]), z.dtype)
            for z in zero_outs
        ]
        out_arrs = fn(*concat_in, *concat_zero)
        return [
            {name: np.asarray(out_arrs[i]).reshape(N_CORES, *out_avals[i].shape)[c]
             for i, name in enumerate(out_names)}
            for c in range(N_CORES)
        ]

    _CACHE["runner"] = run
    return run


def kernel(keys, values, masks, q_in, q_out):
    import ml_dtypes
    f8 = ml_dtypes.float8_e4m3

    keys = np.ascontiguousarray(np.asarray(keys, dtype=np.float32))
    values = np.asarray(values, dtype=np.float32)
    masks = np.asarray(masks, dtype=np.float32)
    q_in = np.ascontiguousarray(np.asarray(q_in, dtype=np.float32))
    q_out = np.ascontiguousarray(np.asarray(q_out, dtype=np.float32))

    # Host-side layout prep (per object, shared by 2 cores)
    # keys8[o]: [64, NB, 2, P]; keys8[o][p, c, i, b] = keys[o, 64i+p, 128c+b]
    keys_pad = np.zeros((OBJ_N, D_KEY, B_PAD), dtype=np.float32)
    keys_pad[:, :, :BANK_N] = keys
    keys8 = np.ascontiguousarray(
        keys_pad.reshape(OBJ_N, 2, 64, NB, P).transpose(0, 2, 3, 1, 4)
    ).astype(f8)
    # vhat8[o]: [P, NB, VHAT_W]; vhat8[o][p, c, v] = vhats[o, 128c+p, v]
    vhats = np.zeros((OBJ_N, B_PAD, VHAT_WP), dtype=np.float32)
    for o in range(OBJ_N):
        vhats[o, :BANK_N, :D_VAL] = values[o].T
        vhats[o, :BANK_N, D_VAL] = masks[o, 0]
        vhats[o, :BANK_N, D_VAL + 1] = 1.0
    vhat8 = np.ascontiguousarray(
        vhats.reshape(OBJ_N, NB, P, VHAT_WP).transpose(0, 2, 1, 3)
    ).astype(f8)
    # qin8: [64, 2, N_Q] (sliced per half); qin8[p, i, n] = q_in[0, 64i+p, n]
    qin8 = np.ascontiguousarray(
        q_in[0].reshape(2, 64, N_Q).transpose(1, 0, 2)
    ).astype(f8)

    consts = np.zeros((2, P), dtype=np.float32)
    consts[1, :] = 1.0
    ebias_arr = np.full((P, 1), EXP_BIAS, dtype=np.float32)

    in_maps = []
    for core in range(N_CORES):
        o, half = divmod(core, 2)
        nsl = slice(half * N_HALF, (half + 1) * N_HALF)
        in_maps.append({
            "consts": consts,
            "ebias": ebias_arr,
            "keys": keys8[o],
            "vhat": vhat8[o],
            "qin": np.ascontiguousarray(qin8[:, :, nsl]),
            "qout": np.ascontiguousarray(q_out[0, :, nsl]),
        })

    run = _get_runner()
    results = run(in_maps)

    out = np.empty((1, OBJ_N, 2 * D_VAL, N_Q), dtype=np.float32)
    for core in range(N_CORES):
        o, half = divmod(core, 2)
        nsl = slice(half * N_HALF, (half + 1) * N_HALF)
        out[0, o, :, nsl] = results[core]["out"]
    return out


# revision 22
# speedup vs baseline: 1.9301x; 1.9301x over previous
"""Trainium2 Bass kernel for nn_Matcher (retrieval_knn attention), fp8 edition.

Math (per object o, with S=1 batch):
  logits[b,n] = (keys[o,:,b] . q_in[:,n]) / sqrt(Dk)
  p           = softmax_b(logits)
  mem[v,n]    = sum_b values[o,v,b] p[b,n]
  maskmem[n]  = sum_b masks[o,b] p[b,n]
  out[o]      = concat([mem, q_out * maskmem], axis=0)   # [1024, n]

Sharding: 8 cores = 4 objects x 2 query halves (n in [0,1800) / [1800,3600)).
No cross-core communication.

Performance design (vs the fp32r baseline):
  * All matmuls run in fp8 e4m3 with MatmulPerfMode.DoubleRow: each PE
    instruction contracts TWO 128-deep k-tiles at 0.5 cycles per output
    column (2 fp8 weights per PE cell) -- 4x the fp32r matmul rate.
      - mm1 (logits): d=128 contraction split as two 64-deep tiles.
      - mm2 (values/mask/denominator): bank chunks processed in pairs.
  * exp on ACT is then the bottleneck: it is fused over chunk PAIRS
    ([128,2,nw] PSUM -> fp8 SBUF in one instruction) to amortize the
    per-instruction SBUF/PSUM access overhead.  The fp8 exp output feeds
    mm2 directly.
  * Accuracy: the output norm is dominated by the q_out*maskmem half
    (entries ~0.5 std vs ~0.02 std for the mem half), and maskmem/denom
    are positive-weighted averages where fp8 noise averages down; measured
    rel err is ~2e-3 against the 2e-2 gate.
  * exp is computed as exp(s/sqrt(Dk) - 2) to keep values in e4m3 range;
    the uniform e^-2 factor cancels exactly in the softmax normalization.

Per n-chunk (nw<=512) pipeline, ACT-paced, PSUM = 8 banks exactly:
  s-pairs (2 tiles x 2 banks, double buffered)  mm1 -> exp
  accA0, accA1 (mem rows 0..255)  + md (maskraw+denom, M=2)  accumulate
    per pair, chasing exp;
  accB (1 bank): mem rows 256..383 then 384..511 as two sub-sweeps that
    re-read the previous n-chunk's fp8 exp tiles, interleaved into the
    next chunk's pair loop so ACT never stalls.
"""

import sys

sys.path.insert(0, "/opt/trn_rl_repo")

import numpy as np

OBJ_N, D_KEY, D_VAL, BANK_N, N_Q = 4, 128, 512, 7200, 3600
N_CORES = 8
N_HALF = N_Q // 2            # 1800 queries per core
P = 128
NB = 58                      # bank chunks, padded: 58*128 = 7424
B_PAD = NB * P
NPAIR = NB // 2              # 29
VHAT_W = D_VAL + 2           # 514: values^T | mask | ones
VHAT_WP = 576                # padded: dual-fp8 ldweights needs 64B-aligned
                             # k-tile stride (512 ok, 514 rejected by walrus)
SCALE = 1.0 / float(np.sqrt(D_KEY))
EXP_BIAS = -2.0              # exp(s*SCALE - 2): cancels in normalization
N_CHUNKS = [(0, 512), (512, 512), (1024, 512), (1536, 264)]  # sum = 1800
NJ = len(N_CHUNKS)

_CACHE = {}


def _build(reps=1, bench=False, reload_in_rep=True):
    import concourse.bacc as bacc
    import concourse.mybir as mybir
    import concourse.tile as tile

    f32 = mybir.dt.float32
    f32r = mybir.dt.float32r
    f8 = mybir.dt.float8e4
    Exp = mybir.ActivationFunctionType.Exp
    DR = mybir.MatmulPerfMode.DoubleRow

    nc = bacc.Bacc("TRN2", target_bir_lowering=False, debug=False)

    ikind = {} if bench else {"kind": "ExternalInput"}
    okind = {} if bench else {"kind": "ExternalOutput"}
    consts_d = nc.dram_tensor("consts", [2, P], f32r, kind="ExternalInput")
    keys_d = nc.dram_tensor("keys", [64, NB, 2, P], f8, **ikind)
    vhat_d = nc.dram_tensor("vhat", [P, NB, VHAT_WP], f8, **ikind)
    qin_d = nc.dram_tensor("qin", [64, 2, N_HALF], f8, **ikind)
    qout_d = nc.dram_tensor("qout", [D_VAL, N_HALF], f32, **ikind)
    out_d = nc.dram_tensor("out", [2 * D_VAL, N_HALF], f32, **okind)
    if bench:
        dout_d = nc.dram_tensor("dout", [1, P], f32, kind="ExternalOutput")

    qout_ap = qout_d.ap().rearrange("(c p) n -> p c n", p=P)        # [128, 4, 1800]
    out_ap = out_d.ap().rearrange("(r p) n -> p r n", p=P)          # [128, 8, 1800]

    with tile.TileContext(nc) as tc:
        with (
            tc.tile_pool(name="persist", bufs=1) as persist,
            tc.tile_pool(name="qin_p", bufs=2) as qin_p,
            tc.tile_pool(name="qout_p", bufs=2) as qout_p,
            tc.tile_pool(name="e_p", bufs=2 * NPAIR) as e_p,
            tc.tile_pool(name="row_p", bufs=2) as row_p,
            tc.tile_pool(name="bcsb_p", bufs=2) as bcsb_p,
            tc.tile_pool(name="out_p", bufs=2) as out_p,
            tc.tile_pool(name="s_ps", bufs=2, space="PSUM") as s_ps,
            tc.tile_pool(name="accA_ps", bufs=1, space="PSUM") as accA_ps,
            tc.tile_pool(name="accB_ps", bufs=1, space="PSUM") as accB_ps,
            tc.tile_pool(name="mdbc_ps", bufs=1, space="PSUM") as mdbc_ps,
        ):
            # Persistent operands
            keys_sb = persist.tile([64, NB, 2, P], f8)
            vhat_sb = persist.tile([P, NB, VHAT_WP], f8)
            ebias = persist.tile([P, 1], f32)
            nc.gpsimd.memset(ebias[:], EXP_BIAS)
            # Warm the ACT exp table so the first real exp doesn't pay the
            # ~2.7us ACT_TABLE_LOAD on the critical path.
            warm = persist.tile([1, 1], f32)
            nc.vector.memset(warm[:], 0.0)
            nc.scalar.activation(warm[:], warm[:], Exp, scale=1.0)
            sel2 = persist.tile([2, P], f32r)  # row0=0, row1=1 (selects denom)
            ones_sb = persist.tile([1, P], f32r)  # row of ones (mask broadcast)
            ones_col = ones_sb[:]
            consts_loaded = [False]

            def bulk_load(first_rep):
                # SP HWDGE queue: qin first (mm1-critical), qout later.
                # Pool SWDGE queue: keys/vhat groups, with the small consts
                # interleaved after the first group so they don't delay the
                # first matmul's data.
                n0_0, nw_0 = N_CHUNKS[0]
                qin_t0 = qin_p.tile([64, 2, nw_0], f8, tag="qin", name="qin_t0")
                nc.sync.dma_start(qin_t0[:], qin_d.ap()[:, :, n0_0:n0_0 + nw_0])
                if first_rep:
                    # all keys up-front (0.95 MB total): exp never waits for
                    # mm1 data; vhat (consumed later by the mem/md matmuls,
                    # which may lag) streams in behind it
                    for g0, g1 in ((0, 2), (2, 16), (16, NB)):
                        nc.gpsimd.dma_start(keys_sb[:, g0:g1], keys_d.ap()[:, g0:g1])
                    g0 = 0
                    for gi, gsz in enumerate([4] * NB):
                        if g0 >= NB:
                            break
                        g1 = min(g0 + gsz, NB)
                        nc.gpsimd.dma_start(vhat_sb[:, g0:g1, :],
                                            vhat_d.ap()[:, g0:g1, :])
                        g0 = g1
                        if gi == 0 and not consts_loaded[0]:
                            nc.gpsimd.dma_start(sel2[:], consts_d.ap()[:, :])
                            nc.gpsimd.dma_start(ones_sb[:], consts_d.ap()[1:2, :])
                            consts_loaded[0] = True
                qout_t0 = qout_p.tile([P, D_VAL // P, nw_0], f32,
                                      tag="qout", name="qout_t0")
                return qin_t0, qout_t0

            def mm1_pair(sp, keys_c0, keys_c1, qin_t, nw):
                nc.tensor.matmul(sp[:, 0, :nw], keys_sb[:, keys_c0], qin_t[:],
                                 start=True, stop=True, perf_mode=DR)
                nc.tensor.matmul(sp[:, 1, :nw], keys_sb[:, keys_c1], qin_t[:],
                                 start=True, stop=True, perf_mode=DR)

            HOIST = 2  # exp-pairs of chunk j+1 emitted before chunk j's
                       # normalization block, so ACT never idles across the
                       # chunk boundary (= s_ps bufs)

            for _rep in range(reps):
                qin_t0, qout_t0 = bulk_load(reload_in_rep or _rep == 0)

                # Pipeline state carried across n-chunks
                prev = None  # (e8_tiles, rb, mn, qout_t, n0, nw) of chunk j-1
                nxt = None   # (qin_t, qout_t, e8_tiles) prefetched for chunk j+1

                def emit_front(e8_tiles, qin_t, pp, nw):
                    c0, c1 = 2 * pp, 2 * pp + 1
                    sp = s_ps.tile([P, 2, 512], f32, tag="s", name="sp")
                    e8 = e_p.tile([P, 2, nw], f8, tag="e8", name="e8")
                    if pp == NPAIR - 1:
                        # chunk 57 is pure bank padding (keys=0, vhat=0): skip
                        # its mm1+exp; zero e8 half 1 so the DR matmuls read a
                        # finite value (x0 contribution from the zero vhat)
                        nc.tensor.matmul(sp[:, 0, :nw], keys_sb[:, c0], qin_t[:],
                                         start=True, stop=True, perf_mode=DR)
                        nc.vector.memset(e8[:, 1, :], 0.0)
                        nc.scalar.activation(e8[:, 0, :], sp[:, 0, :nw], Exp,
                                             scale=SCALE, bias=ebias[:])
                    else:
                        mm1_pair(sp, c0, c1, qin_t, nw)
                        nc.scalar.activation(e8[:, :, :], sp[:, :, :nw], Exp,
                                             scale=SCALE, bias=ebias[:])
                    e8_tiles.append(e8)

                FRONT0 = 4  # chunk-0 fronts emitted before the acc loop so
                            # the pair-0 mem matmuls (gated on the vhat DMA)
                            # cannot stall the mm1->exp feed on the in-order PE

                for j in range(NJ):
                    is_real = True
                    if is_real:
                        n0, nw = N_CHUNKS[j]
                        if j == 0:
                            qin_t, qout_t = qin_t0, qout_t0
                            e8_tiles = []
                            for fp in range(FRONT0):
                                emit_front(e8_tiles, qin_t, fp, nw)
                        else:
                            qin_t, qout_t, e8_tiles = nxt
                            nxt = None
                        accA = [accA_ps.tile([P, nw], f32, tag=f"accA{m}",
                                             name=f"accA{m}") for m in (0, 1)]
                        md_acc = mdbc_ps.tile([2, nw], f32, tag="mdbc", name="md_acc")

                    if prev is not None:
                        pe8, prb, pmn, pqout_t, pn0, pnw = prev
                        accB2 = accB_ps.tile([P, pnw], f32, tag="accB", name="accB2")

                    # ---- pair loop ----
                    for pp in range(NPAIR):
                        c0, c1 = 2 * pp, 2 * pp + 1
                        if is_real:
                            if (pp >= FRONT0 if j == 0 else pp >= HOIST):
                                emit_front(e8_tiles, qin_t, pp, nw)
                            if pp == 10 and j == 0:
                                nc.sync.dma_start(
                                    qout_t[:], qout_ap[:, :, n0:n0 + nw])
                            if pp == 20 and j + 1 < NJ:
                                # prefetch next chunk's query tiles
                                nn0, nnw = N_CHUNKS[j + 1]
                                nqin = qin_p.tile([64, 2, nnw], f8, tag="qin")
                                nc.sync.dma_start(
                                    nqin[:], qin_d.ap()[:, :, nn0:nn0 + nnw])
                                nqout = qout_p.tile([P, D_VAL // P, nnw], f32,
                                                    tag="qout")
                                nc.sync.dma_start(
                                    nqout[:], qout_ap[:, :, nn0:nn0 + nnw])
                                nxt = (nqin, nqout, [])
                            e8 = e8_tiles[pp]
                            nc.tensor.matmul(
                                md_acc[:, :], vhat_sb[:, c0:c0 + 2, D_VAL:D_VAL + 2],
                                e8[:, :, :], start=(pp == 0), stop=(pp == NPAIR - 1),
                                perf_mode=DR)
                            for m in (0, 1):
                                nc.tensor.matmul(
                                    accA[m][:, :],
                                    vhat_sb[:, c0:c0 + 2, m * P:(m + 1) * P],
                                    e8[:, :, :],
                                    start=(pp == 0), stop=(pp == NPAIR - 1),
                                    perf_mode=DR)
                        if prev is not None:
                            # sweep B of chunk j-1, compressed to 4 matmuls per
                            # pair: mem rows 256..383 during pairs 0..7, rows
                            # 384..511 during pairs 8..15.  Finishing early
                            # frees the accB bank so the LAST chunk can run its
                            # own B2 sweep inside this loop (avoiding a long
                            # un-overlapped tail after the final exp).
                            if pp < 8:
                                for c in range(4 * pp, min(4 * pp + 4, NPAIR)):
                                    nc.tensor.matmul(
                                        accB2[:, :],
                                        vhat_sb[:, 2 * c:2 * c + 2, 2 * P:3 * P],
                                        pe8[c][:, :, :],
                                        start=(c == 0), stop=(c == NPAIR - 1),
                                        perf_mode=DR)
                                if pp == 7:
                                    o_t = out_p.tile([P, pnw], f32, tag="out")
                                    nc.vector.tensor_mul(o_t[:], accB2[:], prb[:])
                                    nc.sync.dma_start(
                                        out_ap[:, 2, pn0:pn0 + pnw], o_t[:])
                                    accB3 = accB_ps.tile([P, pnw], f32, tag="accB",
                                                         name="accB3")
                            elif pp < 16:
                                for c in range(4 * (pp - 8), min(4 * (pp - 8) + 4,
                                                                 NPAIR)):
                                    nc.tensor.matmul(
                                        accB3[:, :],
                                        vhat_sb[:, 2 * c:2 * c + 2, 3 * P:4 * P],
                                        pe8[c][:, :, :],
                                        start=(c == 0), stop=(c == NPAIR - 1),
                                        perf_mode=DR)
                                if pp == 15:
                                    o_t = out_p.tile([P, pnw], f32, tag="out")
                                    nc.vector.tensor_mul(o_t[:], accB3[:], prb[:])
                                    nc.sync.dma_start(
                                        out_ap[:, 3, pn0:pn0 + pnw], o_t[:])
                            elif is_real and j == NJ - 1 and pp >= 17:
                                # last chunk: own B2 sweep inline, trailing the
                                # exp stream (only pairs already exp'd: c <= pp)
                                if pp == 17:
                                    accB2o = accB_ps.tile([P, nw], f32, tag="accB",
                                                          name="accB2o")
                                    b2o_cur = 0
                                for c in range(b2o_cur,
                                               min(b2o_cur + 3, pp + 1, NPAIR)):
                                    nc.tensor.matmul(
                                        accB2o[:, :],
                                        vhat_sb[:, 2 * c:2 * c + 2, 2 * P:3 * P],
                                        e8_tiles[c][:, :, :],
                                        start=(c == 0), stop=(c == NPAIR - 1),
                                        perf_mode=DR)
                                b2o_cur = min(b2o_cur + 3, pp + 1, NPAIR)

                    # ---- post-pair block ----
                    if j + 1 < NJ:
                        # hoisted fronts of chunk j+1: keep ACT busy while
                        # the normalization block below stalls PE briefly
                        nnw = N_CHUNKS[j + 1][1]
                        for hp in range(HOIST):
                            emit_front(nxt[2], nxt[0], hp, nnw)
                    # Normalization: md_acc rows 0/1 = [maskraw, denom].
                    md2 = row_p.tile([2, nw], f32r, tag="md2")
                    nc.vector.tensor_copy(md2[:], md_acc[0:2, :])
                    db = mdbc_ps.tile([P, nw], f32, tag="mdbc", name="db_ps")
                    nc.tensor.matmul(db[:], sel2[:], md2[:], start=True, stop=True)
                    rb = bcsb_p.tile([P, nw], f32, tag="rb")
                    nc.vector.reciprocal(rb[:], db[:])

                    mb = mdbc_ps.tile([P, nw], f32, tag="mdbc", name="mb_ps")
                    nc.tensor.matmul(mb[:], ones_col, md2[0:1, :],
                                     start=True, stop=True)

                    if j == NJ - 1:
                        # drain the inline B2 sweep as soon as rb exists
                        o_t = out_p.tile([P, nw], f32, tag="out")
                        nc.vector.tensor_mul(o_t[:], accB2o[:], rb[:])
                        nc.sync.dma_start(out_ap[:, 2, n0:n0 + nw], o_t[:])

                    # drain sweep A accs (mem rows 0..255): one fused DMA
                    st01 = out_p.tile([P, 2, nw], f32, tag="st01")
                    for m in (0, 1):
                        nc.vector.tensor_mul(st01[:, m, :], accA[m][:], rb[:])
                    nc.sync.dma_start(out_ap[:, 0:2, n0:n0 + nw], st01[:])

                    if j == NJ - 1:
                        # B3 sweep of the last chunk in accA0's slot (freed by
                        # the st01 drain above): runs on PE in parallel with
                        # the DVE mn/st47 path below
                        accB3o = accA_ps.tile([P, nw], f32, tag="accA0",
                                              name="accB3o")
                        for c in range(NPAIR):
                            nc.tensor.matmul(
                                accB3o[:, :],
                                vhat_sb[:, 2 * c:2 * c + 2, 3 * P:4 * P],
                                e8_tiles[c][:, :, :],
                                start=(c == 0), stop=(c == NPAIR - 1),
                                perf_mode=DR)

                    mn = bcsb_p.tile([P, nw], f32, tag="mn")
                    nc.vector.tensor_mul(mn[:], mb[:], rb[:])
                    # q_out * maskmem rows: two 2-row DMAs (first transfer
                    # overlaps the second pair's muls)
                    st47 = out_p.tile([P, 4, nw], f32, tag="st47")
                    for m in range(2):
                        nc.vector.tensor_mul(st47[:, m, :], qout_t[:, m, :], mn[:])
                    nc.sync.dma_start(out_ap[:, 4:6, n0:n0 + nw], st47[:, 0:2, :])
                    for m in (2, 3):
                        nc.vector.tensor_mul(st47[:, m, :], qout_t[:, m, :], mn[:])
                    nc.sync.dma_start(out_ap[:, 6:8, n0:n0 + nw], st47[:, 2:4, :])

                    if j == NJ - 1:
                        o_t = out_p.tile([P, nw], f32, tag="out")
                        nc.vector.tensor_mul(o_t[:], accB3o[:], rb[:])
                        nc.sync.dma_start(out_ap[:, 3, n0:n0 + nw], o_t[:])
                        prev = None
                    else:
                        prev = (e8_tiles, rb, mn, qout_t, n0, nw)

            if bench:
                dsb = persist.tile([1, P], f32)
                nc.vector.tensor_copy(dsb[:], ones_sb[:])
                nc.sync.dma_start(dout_d.ap()[:, :], dsb[:])

    nc.compile()
    return nc


def _get_nc():
    if "nc" not in _CACHE:
        _CACHE["nc"] = _build()
    return _CACHE["nc"]


def _get_runner():
    """Build the multi-core PJRT runner once (mirrors bass2jax.run_bass_via_pjrt)."""
    if "runner" in _CACHE:
        return _CACHE["runner"]
    import jax
    from jax.sharding import Mesh, PartitionSpec
    from jax.experimental.shard_map import shard_map
    import concourse.mybir as mybir
    from concourse import bass2jax
    from concourse.bass2jax import _bass_exec_p, install_neuronx_cc_hook

    nc = _get_nc()
    install_neuronx_cc_hook()
    partition_name = nc.partition_id_tensor.name if nc.partition_id_tensor else None
    in_names, out_names, out_avals = [], [], []
    for alloc in nc.m.functions[0].allocations:
        if not isinstance(alloc, mybir.MemoryLocationSet):
            continue
        name = alloc.memorylocations[0].name
        if alloc.kind == "ExternalInput":
            if name != partition_name:
                in_names.append(name)
        elif alloc.kind == "ExternalOutput":
            out_names.append(name)
            out_avals.append(jax.core.ShapedArray(
                tuple(alloc.tensor_shape), mybir.dt.np(alloc.dtype)))
    n_params = len(in_names)
    zero_outs = [np.zeros(a.shape, a.dtype) for a in out_avals]
    all_in_names = list(in_names) + list(out_names)
    if partition_name is not None:
        all_in_names.append(partition_name)

    def _body(*args):
        operands = list(args)
        if partition_name is not None:
            operands.append(bass2jax.partition_id_tensor())
        outs = _bass_exec_p.bind(
            *operands,
            out_avals=tuple(out_avals),
            in_names=tuple(all_in_names),
            out_names=tuple(out_names),
            lowering_input_output_aliases=(),
            sim_require_finite=True,
            sim_require_nnan=True,
            nc=nc,
        )
        return tuple(outs)

    try:
        devices = jax.devices("axon")
    except Exception:
        devices = [d for d in jax.devices() if d.platform != "cpu"] or jax.devices()
    devices = devices[:N_CORES]
    assert len(devices) >= N_CORES, f"need {N_CORES} cores, got {len(devices)}"
    mesh = Mesh(np.asarray(devices), ("core",))
    n_io = n_params + len(out_names)
    fn = jax.jit(
        shard_map(_body, mesh=mesh,
                  in_specs=(PartitionSpec("core"),) * n_io,
                  out_specs=(PartitionSpec("core"),) * len(out_names),
                  check_rep=False),
        keep_unused=True)

    def run(in_maps):
        concat_in = [
            np.concatenate([np.asarray(m[name]) for m in in_maps], axis=0)
            for name in in_names
        ]
        concat_zero = [
            np.zeros((N_CORES * z.shape[0], *z.shape[1:]), z.dtype)
            for z in zero_outs
        ]
        out_arrs = fn(*concat_in, *concat_zero)
        return [
            {name: np.asarray(out_arrs[i]).reshape(N_CORES, *out_avals[i].shape)[c]
             for i, name in enumerate(out_names)}
            for c in range(N_CORES)
        ]

    _CACHE["runner"] = run
    return run


def kernel(keys, values, masks, q_in, q_out):
    import ml_dtypes
    f8 = ml_dtypes.float8_e4m3

    keys = np.ascontiguousarray(np.asarray(keys, dtype=np.float32))
    values = np.asarray(values, dtype=np.float32)
    masks = np.asarray(masks, dtype=np.float32)
    q_in = np.ascontiguousarray(np.asarray(q_in, dtype=np.float32))
    q_out = np.ascontiguousarray(np.asarray(q_out, dtype=np.float32))

    # Host-side layout prep (per object, shared by 2 cores)
    # keys8[o]: [64, NB, 2, P]; keys8[o][p, c, i, b] = keys[o, 64i+p, 128c+b]
    keys_pad = np.zeros((OBJ_N, D_KEY, B_PAD), dtype=np.float32)
    keys_pad[:, :, :BANK_N] = keys
    keys8 = np.ascontiguousarray(
        keys_pad.reshape(OBJ_N, 2, 64, NB, P).transpose(0, 2, 3, 1, 4)
    ).astype(f8)
    # vhat8[o]: [P, NB, VHAT_W]; vhat8[o][p, c, v] = vhats[o, 128c+p, v]
    vhats = np.zeros((OBJ_N, B_PAD, VHAT_WP), dtype=np.float32)
    for o in range(OBJ_N):
        vhats[o, :BANK_N, :D_VAL] = values[o].T
        vhats[o, :BANK_N, D_VAL] = masks[o, 0]
        vhats[o, :BANK_N, D_VAL + 1] = 1.0
    vhat8 = np.ascontiguousarray(
        vhats.reshape(OBJ_N, NB, P, VHAT_WP).transpose(0, 2, 1, 3)
    ).astype(f8)
    # qin8: [64, 2, N_Q] (sliced per half); qin8[p, i, n] = q_in[0, 64i+p, n]
    qin8 = np.ascontiguousarray(
        q_in[0].reshape(2, 64, N_Q).transpose(1, 0, 2)
    ).astype(f8)

    consts = np.zeros((2, P), dtype=np.float32)
    consts[1, :] = 1.0

    in_maps = []
    for core in range(N_CORES):
        o, half = divmod(core, 2)
        nsl = slice(half * N_HALF, (half + 1) * N_HALF)
        in_maps.append({
            "consts": consts,
            "keys": keys8[o],
            "vhat": vhat8[o],
            "qin": np.ascontiguousarray(qin8[:, :, nsl]),
            "qout": np.ascontiguousarray(q_out[0, :, nsl]),
        })

    run = _get_runner()
    results = run(in_maps)

    out = np.empty((1, OBJ_N, 2 * D_VAL, N_Q), dtype=np.float32)
    for core in range(N_CORES):
        o, half = divmod(core, 2)
        nsl = slice(half * N_HALF, (half + 1) * N_HALF)
        out[0, o, :, nsl] = results[core]["out"]
    return out
